# revision 9
# baseline (speedup 1.0000x reference)
"""Multi-head attention (B=2, N=4096, C=512, H=8, D=64) on 8 TRN2 NeuronCores.

Sharding: core c handles batch b = c // 4 and head-pair p = c % 4
(heads 2p, 2p+1, i.e. channels [128p, 128p+128) of the QKV projections).
Each core computes a partial output projection O_loc @ Wo_loc; the host
sums the 4 partials per batch and adds bo.

Device dataflow per core (all matmuls bf16 with fp32 PSUM accumulate):
  - xT (pre-transposed on host, bf16 [C=512, N=4096]) streams in.
  - Q^T, K^T = W_loc^T @ xT + bias  ([128 hd, 4096 pos], bias per-partition)
  - V      = x @ Wv_loc + bv        ([4096 pos, 128 hd], bias via ones-row matmul)
    V_aug: per head [pos, 65] tiles: 64 V columns + ones column (softmax denom).
  - For each q-block of 512 and key-chunk of 128:
      S^T chunk = K^T_chunk.T @ Q^T_block  -> PSUM [128 keys, 512 q] per head
      P = exp(S^T / 8)  (ScalarE, both heads in one [128, 1024] instr, bf16 out)
      PV: P^T_chunk as stationary, V_aug chunk as moving -> O/denominator
          accumulate in PSUM [q, 65] per q-subtile of 128.
  - Normalize O by the denominator (VectorE reciprocal + per-partition mul),
    PE-transpose to O^T, output projection O @ Wo_loc -> partial out rows.
No max-subtraction in softmax: scores/8 are bounded (|s|<~3) for this
problem's input distribution, so exp is safe in fp32/bf16.
"""

import numpy as np
import ml_dtypes

import concourse.bass as bass
import concourse.mybir as mybir
import concourse.tile as tile
from concourse import bacc
from concourse.bass_utils import run_bass_kernel_spmd
from concourse.masks import make_identity

F32 = mybir.dt.float32
BF16 = mybir.dt.bfloat16
AF = mybir.ActivationFunctionType

N = 4096
C = 512
HD = 128          # channels per core (2 heads x 64)
D = 64
QB = 512          # q-block
NQB = N // QB     # 8
KC = 128          # key chunk
NKC = N // KC     # 32
PVW = 66          # padded stride for [O(64) | denom(1)] subtiles in PSUM


def build_nc(debug=False):
    nc = bacc.Bacc(None, target_bir_lowering=False)

    xT = nc.declare_dram_parameter("xT", [C, N], BF16, isOutput=False)
    wq = nc.declare_dram_parameter("wq", [C, HD], BF16, isOutput=False)
    wk = nc.declare_dram_parameter("wk", [C, HD], BF16, isOutput=False)
    wv = nc.declare_dram_parameter("wv", [C, HD], BF16, isOutput=False)
    wo = nc.declare_dram_parameter("wo", [HD, C], BF16, isOutput=False)
    bq = nc.declare_dram_parameter("bq", [HD, 1], F32, isOutput=False)
    bk = nc.declare_dram_parameter("bk", [HD, 1], F32, isOutput=False)
    bv = nc.declare_dram_parameter("bv", [1, HD], BF16, isOutput=False)
    out = nc.declare_dram_parameter("out", [N, C], F32, isOutput=True)
    if debug:
        dbg = {
            "qt": nc.declare_dram_parameter("d_qt", [HD, N], BF16, isOutput=True),
            "kt": nc.declare_dram_parameter("d_kt", [HD, N], BF16, isOutput=True),
            "va0": nc.declare_dram_parameter("d_va0", [128, NKC * 65], BF16, isOutput=True),
            "va1": nc.declare_dram_parameter("d_va1", [128, NKC * 65], BF16, isOutput=True),
            "p00": nc.declare_dram_parameter("d_p00", [128, 2 * QB], BF16, isOutput=True),
            "pv0": nc.declare_dram_parameter("d_pv0", [128, 4 * PVW], F32, isOutput=True),
            "pv1": nc.declare_dram_parameter("d_pv1", [128, 4 * PVW], F32, isOutput=True),
            "o2t": nc.declare_dram_parameter("d_o2t", [HD, QB], BF16, isOutput=True),
        }

    with tile.TileContext(nc) as tc:
        with (
            tc.tile_pool(name="const", bufs=1) as cpool,
            tc.tile_pool(name="big", bufs=1) as bpool,
        ):
            # Constants / weights in SBUF
            xt = [cpool.tile([128, N], BF16, tag=f"xt{c}", name=f"xt{c}") for c in range(4)]
            wq_s = cpool.tile([128, C], BF16, tag="wq")
            wk_s = cpool.tile([128, C], BF16, tag="wk")
            wv_s = cpool.tile([128, C], BF16, tag="wv")
            wo_s = cpool.tile([HD, C], BF16, tag="wo")
            bq_s = cpool.tile([HD, 1], F32, tag="bq")
            bk_s = cpool.tile([HD, 1], F32, tag="bk")
            bv_s = cpool.tile([1, HD], BF16, tag="bv")
            ones_s = cpool.tile([1, 128], BF16, tag="ones")
            ident = cpool.tile([128, 128], BF16, tag="ident")

            for c in range(4):
                nc.sync.dma_start(out=xt[c][:], in_=xT[c * 128:(c + 1) * 128, :])
                nc.sync.dma_start(out=wq_s[:, c * 128:(c + 1) * 128],
                                  in_=wq[c * 128:(c + 1) * 128, :])
                nc.sync.dma_start(out=wk_s[:, c * 128:(c + 1) * 128],
                                  in_=wk[c * 128:(c + 1) * 128, :])
                nc.sync.dma_start(out=wv_s[:, c * 128:(c + 1) * 128],
                                  in_=wv[c * 128:(c + 1) * 128, :])
            nc.sync.dma_start(out=wo_s[:], in_=wo[:])
            nc.sync.dma_start(out=bq_s[:], in_=bq[:])
            nc.sync.dma_start(out=bk_s[:], in_=bk[:])
            nc.sync.dma_start(out=bv_s[:], in_=bv[:])
            nc.vector.memset(ones_s[:], 1.0)
            make_identity(nc, ident[:])

            # Persistent activations
            qt = bpool.tile([HD, N], BF16, tag="qt")
            kt = bpool.tile([HD, N], BF16, tag="kt")
            vaug = [bpool.tile([128, NKC * 65], BF16, tag=f"vaug{h}", name=f"vaug{h}") for h in (0, 1)]
            nc.vector.memset(vaug[0][:], 1.0)
            nc.vector.memset(vaug[1][:], 1.0)

            # ---- Projections ----
            with tc.tile_pool(name="pp", bufs=4, space="PSUM") as pp:
                for qb in range(NQB):
                    sl = slice(qb * QB, (qb + 1) * QB)
                    pq = pp.tile([128, QB], F32, tag="pj")
                    for c in range(4):
                        nc.tensor.matmul(pq[:], lhsT=wq_s[:, c * 128:(c + 1) * 128],
                                         rhs=xt[c][:, sl],
                                         start=(c == 0), stop=(c == 3))
                    nc.vector.tensor_scalar(out=qt[:, sl], in0=pq[:],
                                            scalar1=bq_s[:], scalar2=None,
                                            op0=mybir.AluOpType.add)
                    pk = pp.tile([128, QB], F32, tag="pj")
                    for c in range(4):
                        nc.tensor.matmul(pk[:], lhsT=wk_s[:, c * 128:(c + 1) * 128],
                                         rhs=xt[c][:, sl],
                                         start=(c == 0), stop=(c == 3))
                    nc.vector.tensor_scalar(out=kt[:, sl], in0=pk[:],
                                            scalar1=bk_s[:], scalar2=None,
                                            op0=mybir.AluOpType.add)
                for pt in range(NKC):
                    psl = slice(pt * 128, (pt + 1) * 128)
                    pv = pp.tile([128, 128], F32, tag="pj")
                    for c in range(4):
                        nc.tensor.matmul(pv[:], lhsT=xt[c][:, psl],
                                         rhs=wv_s[:, c * 128:(c + 1) * 128],
                                         start=(c == 0), stop=False)
                    nc.tensor.matmul(pv[:], lhsT=ones_s[:], rhs=bv_s[:],
                                     start=False, stop=True)
                    for h in (0, 1):
                        nc.vector.tensor_copy(
                            out=vaug[h][:, pt * 65:pt * 65 + 64],
                            in_=pv[:, h * 64:(h + 1) * 64])

            if debug:
                nc.sync.dma_start(out=dbg["qt"][:], in_=qt[:])
                nc.sync.dma_start(out=dbg["kt"][:], in_=kt[:])
                nc.sync.dma_start(out=dbg["va0"][:], in_=vaug[0][:])
                nc.sync.dma_start(out=dbg["va1"][:], in_=vaug[1][:])

            # ---- Attention + output projection ----
            with (
                tc.tile_pool(name="sps", bufs=2, space="PSUM") as sps,
                tc.tile_pool(name="pvp", bufs=1, space="PSUM") as pvp,
                tc.tile_pool(name="trp", bufs=1, space="PSUM") as trp,
                tc.tile_pool(name="opp", bufs=1, space="PSUM") as opp,
                tc.tile_pool(name="ptp", bufs=3) as ptp,
                tc.tile_pool(name="msc", bufs=4) as msc,
                tc.tile_pool(name="o2p", bufs=2) as o2p,
                tc.tile_pool(name="obp", bufs=3) as obp,
            ):
                for qb in range(NQB):
                    qsl = slice(qb * QB, (qb + 1) * QB)
                    pv_ps = [pvp.tile([128, 4 * PVW], F32, tag=f"pv{h}", name=f"pv{h}")
                             for h in (0, 1)]

                    def s_mm(kc):
                        s = sps.tile([128, 2 * QB], F32, tag="s")
                        for h in (0, 1):
                            hsl = slice(h * D, (h + 1) * D)
                            nc.tensor.matmul(
                                s[:, h * QB:(h + 1) * QB],
                                lhsT=kt[hsl, kc * KC:(kc + 1) * KC],
                                rhs=qt[hsl, qsl], start=True, stop=True)
                        return s

                    s_cur = s_mm(0)
                    for kc in range(NKC):
                        s_next = s_mm(kc + 1) if kc + 1 < NKC else None
                        p = ptp.tile([128, 2 * QB], BF16, tag="p")
                        nc.scalar.activation(p[:], s_cur[:], AF.Exp, scale=0.125)
                        if debug and qb == 0 and kc == 0:
                            nc.sync.dma_start(out=dbg["p00"][:], in_=p[:])
                        for h in (0, 1):
                            for qs in range(4):
                                # start=True clears the whole PSUM bank, so only
                                # the first matmul into this tile may use it.
                                nc.tensor.matmul(
                                    pv_ps[h][:, qs * PVW:qs * PVW + 65],
                                    lhsT=p[:, h * QB + qs * 128:h * QB + (qs + 1) * 128],
                                    rhs=vaug[h][:, kc * 65:(kc + 1) * 65],
                                    start=(kc == 0 and qs == 0), stop=(kc == NKC - 1))
                        s_cur = s_next

                    # normalize + transpose to O^T
                    if debug and qb == 0:
                        for h in (0, 1):
                            dcp = obp.tile([128, 4 * PVW], F32, tag="dcp")
                            nc.vector.tensor_copy(out=dcp[:], in_=pv_ps[h][:])
                            nc.sync.dma_start(out=dbg[f"pv{h}"][:], in_=dcp[:])
                    o2t = o2p.tile([HD, QB], BF16, tag="o2t")
                    for h in (0, 1):
                        for qs in range(4):
                            rec = msc.tile([128, 1], F32, tag="rec")
                            nc.vector.reciprocal(
                                rec[:], pv_ps[h][:, qs * PVW + 64:qs * PVW + 65])
                            onrm = msc.tile([128, D], BF16, tag="onrm")
                            nc.vector.tensor_scalar(
                                out=onrm[:], in0=pv_ps[h][:, qs * PVW:qs * PVW + 64],
                                scalar1=rec[:], scalar2=None,
                                op0=mybir.AluOpType.mult)
                            tr = trp.tile([D, 128], BF16, tag="tr")
                            nc.tensor.transpose(tr[:], onrm[:], ident[:])
                            nc.vector.tensor_copy(
                                out=o2t[h * D:(h + 1) * D, qs * 128:(qs + 1) * 128],
                                in_=tr[:])

                    if debug and qb == 0:
                        nc.sync.dma_start(out=dbg["o2t"][:], in_=o2t[:])
                    # output projection for this q-block
                    for qs in range(4):
                        po = opp.tile([128, C], F32, tag="po")
                        nc.tensor.matmul(po[:], lhsT=o2t[:, qs * 128:(qs + 1) * 128],
                                         rhs=wo_s[:], start=True, stop=True)
                        ob = obp.tile([128, C], F32, tag="ob")
                        nc.vector.tensor_copy(out=ob[:], in_=po[:])
                        r0 = qb * QB + qs * 128
                        nc.sync.dma_start(out=out[r0:r0 + 128, :], in_=ob[:])

    nc.compile()
    return nc


_NC_CACHE = {}


def _get_nc():
    if "nc" not in _NC_CACHE:
        _NC_CACHE["nc"] = build_nc()
    return _NC_CACHE["nc"]


def kernel(x, Wq, bq, Wk, bk, Wv, bv, Wo, bo):
    x = np.asarray(x, dtype=np.float32)
    bf = ml_dtypes.bfloat16
    nc = _get_nc()

    in_maps = []
    for c in range(8):
        b, p = c // 4, c % 4
        cs = slice(p * HD, (p + 1) * HD)
        in_maps.append({
            "xT": np.ascontiguousarray(x[b].T).astype(bf),
            "wq": np.ascontiguousarray(Wq[:, cs]).astype(bf),
            "wk": np.ascontiguousarray(Wk[:, cs]).astype(bf),
            "wv": np.ascontiguousarray(Wv[:, cs]).astype(bf),
            "wo": np.ascontiguousarray(Wo[cs, :]).astype(bf),
            "bq": np.asarray(bq[cs], np.float32).reshape(HD, 1).copy(),
            "bk": np.asarray(bk[cs], np.float32).reshape(HD, 1).copy(),
            "bv": np.asarray(bv[cs], np.float32).reshape(1, HD).astype(bf),
        })

    res = run_bass_kernel_spmd(nc, in_maps, core_ids=list(range(8)))

    out = np.zeros((2, N, C), np.float32)
    for c in range(8):
        out[c // 4] += res.results[c]["out"]
    out += np.asarray(bo, np.float32)[None, None, :]
    return out
